# revision 9
# baseline (speedup 1.0000x reference)
"""Trainium2 Bass kernel for nn_EnhancedBaselineWithReturnBoost.

4-layer transformer encoder (D=256, H=8, DI=1024) over [B=256, S=128] location
sequences, final-token head into V=50000 logits, plus a scatter-add "return
boost" on recent locations, ensembled with sigmoid(ensemble_weight).

Sharding: pure data-parallel over batch across 8 NeuronCores (32 batch items
per core).  One batch item = one 128-token tile (S=128).  All activations stay
resident in SBUF; matmuls run in bf16 with fp32 PSUM accumulation; the
residual stream is fp32.

Host-side preprocessing (exact, standard inference folds):
  - LN gains folded into adjacent weight matrices (diag(g) @ W), LN biases
    folded into the following bias vectors (b @ W).
  - Weights cast to bf16 once.
  - Positional encoding table computed (deterministic constant).
  - Boost scatter indices converted to per-core (256-elem block, offset)
    one-hot rows; device applies them with an indirect-DMA read-modify-write
    on the logits in HBM, scaled by sigmoid(w)*return_strength computed
    on-device.
"""
import numpy as np
import ml_dtypes
from contextlib import ExitStack

import concourse.bass as bass
import concourse.mybir as mybir
import concourse.tile as tile
from concourse import bacc, bass_utils
from concourse.bass import IndirectOffsetOnAxis
from concourse.masks import make_identity

F32 = mybir.dt.float32
BF16 = mybir.dt.bfloat16
I32 = mybir.dt.int32
AF = mybir.ActivationFunctionType
ALU = mybir.AluOpType

# problem dims (hardcoded per spec)
V, U, D, DI, L, H, B, S = 50000, 1024, 256, 1024, 4, 8, 256, 128
DK = D // H            # 32
NB = 5                 # boost positions
NCORES = 8
BPC = B // NCORES      # 32 batch items per core
P = 128
KD = D // P            # 2 contraction chunks for D
KI = DI // P           # 8 chunks for DI
NV = 500               # head logits chunk width
NVC = V // NV          # 100 chunks
BLK = 256              # boost scatter block (f32 elems) = 1KB
NBLK = BPC * V // BLK  # 6250 blocks per core
BCAP = 256             # max boost rows per core (>= BPC*NB=160), 2 rounds of 128
ATT_SCALE = 1.0 / np.sqrt(DK)
EPS = 1e-5


def _posenc():
    pos = np.arange(S)[:, None]
    i = np.arange(0, D, 2)[None, :]
    ang = pos / np.power(10000.0, i / D)
    pe = np.zeros((S, D), dtype=np.float32)
    pe[:, 0::2] = np.sin(ang)
    pe[:, 1::2] = np.cos(ang)
    return pe


def _build(with_ln1_bias: bool):
    """Build + compile the per-core Bass program (SPMD: same NEFF, per-core data)."""
    nc = bacc.Bacc("TRN2", target_bir_lowering=False, debug=False, num_devices=NCORES)

    # ---- DRAM I/O -----------------------------------------------------------
    locs_d = nc.dram_tensor("locs", [BPC, S], I32, kind="ExternalInput")
    users_d = nc.dram_tensor("users", [BPC, P], I32, kind="ExternalInput")
    lemb_d = nc.dram_tensor("lemb", [V, D], F32, kind="ExternalInput")
    uemb_d = nc.dram_tensor("uemb", [U, D], BF16, kind="ExternalInput")
    pos_d = nc.dram_tensor("posenc", [S, D], F32, kind="ExternalInput")
    wq_d = nc.dram_tensor("wq", [L, D, D], BF16, kind="ExternalInput")
    wk_d = nc.dram_tensor("wk", [L, D, D], BF16, kind="ExternalInput")
    wv_d = nc.dram_tensor("wv", [L, D, D], BF16, kind="ExternalInput")
    wo_d = nc.dram_tensor("wo", [L, D, D], BF16, kind="ExternalInput")
    w1_d = nc.dram_tensor("w1", [L, D, DI], BF16, kind="ExternalInput")
    w2_d = nc.dram_tensor("w2", [L, DI, D], BF16, kind="ExternalInput")
    b1t_d = nc.dram_tensor("b1t", [L, P, KI], F32, kind="ExternalInput")
    b2r_d = nc.dram_tensor("b2r", [L, 1, D], BF16, kind="ExternalInput")
    lnb_d = nc.dram_tensor("lnbrows", [3, L, 1, D], BF16, kind="ExternalInput")
    wout_d = nc.dram_tensor("wout", [D, V], BF16, kind="ExternalInput")
    bout_d = nc.dram_tensor("bout", [1, V], BF16, kind="ExternalInput")
    ens_d = nc.dram_tensor("ens", [1, 1], F32, kind="ExternalInput")
    rstr_d = nc.dram_tensor("rstr", [1, 1], F32, kind="ExternalInput")
    bidx_d = nc.dram_tensor("bidx", [BCAP, 1], I32, kind="ExternalInput")
    brows_d = nc.dram_tensor("brows", [BCAP, BLK], F32, kind="ExternalInput")
    out_d = nc.dram_tensor("out", [BPC, V], F32, kind="ExternalOutput")

    head_dma_insts = []

    with tile.TileContext(nc) as tc, ExitStack() as ctx:
        cp = ctx.enter_context(tc.tile_pool(name="const", bufs=1))
        wp = ctx.enter_context(tc.tile_pool(name="wts", bufs=1))
        sp = ctx.enter_context(tc.tile_pool(name="work", bufs=3))
        ap_ = ctx.enter_context(tc.tile_pool(name="attw", bufs=4))
        gp = ctx.enter_context(tc.tile_pool(name="gath", bufs=4))
        hp = ctx.enter_context(tc.tile_pool(name="head", bufs=3))
        # PSUM: 8 banks total. psA(3) + psB(3) + psC(2) = 8.
        psA = ctx.enter_context(tc.tile_pool(name="psA", bufs=3, space="PSUM"))
        psB = ctx.enter_context(tc.tile_pool(name="psB", bufs=3, space="PSUM"))
        psC = ctx.enter_context(tc.tile_pool(name="psC", bufs=2, space="PSUM"))

        def p128(dtype=F32):
            return psA.tile([P, P], dtype, tag="ps128", name="p128")

        def p256(dtype=F32, shape=(P, 256)):
            return psB.tile(list(shape), dtype, tag="ps256", name="p256")

        def pmisc(shape, dtype=F32):
            return psC.tile(list(shape), dtype, tag="misc", name="pmisc")

        # ---- constants ------------------------------------------------------
        ident = cp.tile([P, P], BF16)
        make_identity(nc, ident[:])
        ones_r32 = cp.tile([1, P], F32)
        nc.vector.memset(ones_r32[:], 1.0)
        ones_rbf = cp.tile([1, P], BF16)
        nc.vector.memset(ones_rbf[:], 1.0)
        ones_cbf = cp.tile([P, 1], BF16)
        nc.vector.memset(ones_cbf[:], 1.0)
        ones_m32 = cp.tile([P, DK], BF16)
        nc.vector.memset(ones_m32[:], 1.0)

        pos_sb = cp.tile([P, D], F32)
        nc.sync.dma_start(out=pos_sb[:], in_=pos_d[:])
        b1t_sb = cp.tile([P, L * KI], F32)
        for l in range(L):
            nc.sync.dma_start(out=b1t_sb[:, l * KI:(l + 1) * KI], in_=b1t_d[l])
        b2r_sb = cp.tile([1, L * D], BF16)
        for l in range(L):
            nc.sync.dma_start(out=b2r_sb[:, l * D:(l + 1) * D], in_=b2r_d[l])
        lnb_sb = None
        if with_ln1_bias:
            lnb_sb = cp.tile([1, 3 * L * D], BF16)
            for t in range(3):
                for l in range(L):
                    nc.sync.dma_start(
                        out=lnb_sb[:, (t * L + l) * D:(t * L + l + 1) * D],
                        in_=lnb_d[t, l],
                    )

        # ---- weights resident in SBUF --------------------------------------
        wq_sb, wk_sb, wv_sb, wo_sb, w1_sb, w2_sb = [], [], [], [], [], []
        for l in range(L):
            for (lst, dram, width) in (
                (wq_sb, wq_d, D), (wk_sb, wk_d, D), (wv_sb, wv_d, D), (wo_sb, wo_d, D),
            ):
                t = wp.tile([P, KD * width], BF16, tag=f"w{id(lst)}_{l}")
                for kc in range(KD):
                    nc.sync.dma_start(
                        out=t[:, kc * width:(kc + 1) * width],
                        in_=dram[l, kc * P:(kc + 1) * P, :],
                    )
                lst.append(t)
            t = wp.tile([P, KD * DI], BF16, tag=f"w1_{l}")
            for kc in range(KD):
                nc.sync.dma_start(out=t[:, kc * DI:(kc + 1) * DI],
                                  in_=w1_d[l, kc * P:(kc + 1) * P, :])
            w1_sb.append(t)
            t = wp.tile([P, KI * D], BF16, tag=f"w2_{l}")
            for ki in range(KI):
                nc.sync.dma_start(out=t[:, ki * D:(ki + 1) * D],
                                  in_=w2_d[l, ki * P:(ki + 1) * P, :])
            w2_sb.append(t)

        # ---- scalars: w = sigmoid(ens); 1-w ; s = w*rstr --------------------
        ens_sb = cp.tile([1, 1], F32)
        nc.sync.dma_start(out=ens_sb[:], in_=ens_d[:])
        rstr_sb = cp.tile([1, 1], F32)
        nc.sync.dma_start(out=rstr_sb[:], in_=rstr_d[:])
        w_sb = cp.tile([1, 1], F32)
        nc.scalar.activation(out=w_sb[:], in_=ens_sb[:], func=AF.Sigmoid)
        onem_sb = cp.tile([1, 1], F32)
        nc.vector.tensor_scalar(out=onem_sb[:], in0=w_sb[:], scalar1=-1.0,
                                scalar2=1.0, op0=ALU.mult, op1=ALU.add)
        s_sb = cp.tile([1, 1], F32)
        nc.vector.tensor_tensor(out=s_sb[:], in0=w_sb[:], in1=rstr_sb[:], op=ALU.mult)
        # broadcasts via PE (fp32, tiny)
        ps_a = pmisc([BPC, 1])
        nc.tensor.matmul(out=ps_a[:], lhsT=ones_r32[:, 0:BPC], rhs=onem_sb[:],
                         start=True, stop=True)
        onem32 = cp.tile([BPC, 1], F32)
        nc.scalar.copy(out=onem32[:], in_=ps_a[:])
        ps_b = pmisc([P, 1])
        nc.tensor.matmul(out=ps_b[:], lhsT=ones_r32[:], rhs=s_sb[:],
                         start=True, stop=True)
        sbc = cp.tile([P, 1], F32)
        nc.scalar.copy(out=sbc[:], in_=ps_b[:])
        # (1-w) replicated along free dim as bf16 row [1, BPC] for head bias matmul
        onemw_row32 = cp.tile([1, BPC], F32)
        nc.vector.tensor_scalar(out=onemw_row32[:], in0=ones_r32[:, 0:BPC],
                                scalar1=onem_sb[:, 0:1], scalar2=None, op0=ALU.mult)
        onemw_row = cp.tile([1, BPC], BF16)
        nc.vector.tensor_copy(out=onemw_row[:], in_=onemw_row32[:])

        # ---- residual stream ------------------------------------------------
        x_big = cp.tile([P, BPC * D], F32)   # x for all 32 batch tiles

        def ln_apply(xb, out_bf, extra_scale=None):
            """LayerNorm stats + apply: out_bf = (x - mu) * rsqrt(var+eps) [* extra]"""
            st = sp.tile([P, 6], F32, tag="lnst")
            nc.vector.bn_stats(out=st[:xb.shape[0]], in_=xb)
            ag = sp.tile([P, 2], F32, tag="lnag")
            nc.vector.bn_aggr(out=ag[:xb.shape[0], :], in_=st[:xb.shape[0]])
            npart = xb.shape[0]
            rec = sp.tile([P, 1], F32, tag="lnrec")
            nc.vector.tensor_scalar(out=rec[:npart], in0=ag[:npart, 1:2],
                                    scalar1=EPS, scalar2=None, op0=ALU.add)
            nc.vector.reciprocal(out=rec[:npart], in_=rec[:npart])
            rs = sp.tile([P, 1], F32, tag="lnrs")
            nc.scalar.sqrt(out=rs[:npart], in_=rec[:npart])
            if extra_scale is not None:
                nc.vector.tensor_tensor(out=rs[:npart], in0=rs[:npart],
                                        in1=extra_scale, op=ALU.mult)
            nmu = sp.tile([P, 1], F32, tag="lnnmu")
            nc.vector.scalar_tensor_tensor(out=nmu[:npart], in0=ag[:npart, 0:1],
                                           scalar=-1.0, in1=rs[:npart],
                                           op0=ALU.mult, op1=ALU.mult)
            nc.scalar.activation(out=out_bf, in_=xb, func=AF.Identity,
                                 bias=nmu[:npart], scale=rs[:npart])

        def transpose_256(src_bf, dst_bf, copy_engines=(nc.scalar, nc.vector)):
            """[128, 256] bf16 -> [128, 2*128] (chunk-major rows of the transpose)."""
            for c in range(KD):
                pt = p128(BF16)
                nc.tensor.transpose(out=pt[:], in_=src_bf[:, c * P:(c + 1) * P],
                                    identity=ident[:])
                eng = copy_engines[c % len(copy_engines)]
                if eng is nc.scalar:
                    nc.scalar.copy(out=dst_bf[:, c * P:(c + 1) * P], in_=pt[:])
                else:
                    nc.vector.tensor_copy(out=dst_bf[:, c * P:(c + 1) * P], in_=pt[:])

        # ================= per-batch-tile pipeline ==========================
        for b in range(BPC):
            xb = x_big[:, b * D:(b + 1) * D]
            # ---- embedding: x = loc_emb[locs] + user[b] + posenc
            lidx = gp.tile([P, 1], I32, tag="lidx")
            nc.sync.dma_start(out=lidx[:], in_=locs_d[b, :, None])
            xg = gp.tile([P, D], F32, tag="xg")
            nc.gpsimd.indirect_dma_start(
                out=xg[:], out_offset=None, in_=lemb_d[:],
                in_offset=IndirectOffsetOnAxis(ap=lidx[:, :1], axis=0))
            uix = gp.tile([P, 1], I32, tag="uix")
            nc.sync.dma_start(out=uix[:], in_=users_d[b, :, None])
            ub = gp.tile([P, D], BF16, tag="ub")
            nc.gpsimd.indirect_dma_start(
                out=ub[:], out_offset=None, in_=uemb_d[:],
                in_offset=IndirectOffsetOnAxis(ap=uix[:, :1], axis=0))
            nc.vector.tensor_add(out=xb, in0=xg[:], in1=pos_sb[:])
            nc.vector.tensor_add(out=xb, in0=xb, in1=ub[:])

            for l in range(L):
                # ---------- attention ----------
                h1 = sp.tile([P, D], BF16, tag="h1")
                ln_apply(xb, h1[:])
                h1T = sp.tile([P, D], BF16, tag="h1T")
                transpose_256(h1[:], h1T[:])

                qT = sp.tile([P, D], BF16, tag="qT")
                kT = sp.tile([P, D], BF16, tag="kT")
                for (dst, wsb, bi) in ((qT, wq_sb[l], 0), (kT, wk_sb[l], 1)):
                    for m in range(KD):
                        pq = p128(F32)
                        for kc in range(KD):
                            nc.tensor.matmul(
                                out=pq[:],
                                lhsT=wsb[:, kc * D + m * P: kc * D + (m + 1) * P],
                                rhs=h1T[:, kc * P:(kc + 1) * P],
                                start=(kc == 0), stop=(kc == KD - 1 and lnb_sb is None))
                        if lnb_sb is not None:
                            nc.tensor.matmul(
                                out=pq[:],
                                lhsT=lnb_sb[:, (bi * L + l) * D + m * P:
                                            (bi * L + l) * D + (m + 1) * P],
                                rhs=ones_rbf[:], start=False, stop=True)
                        eng = nc.scalar if (m + bi) % 2 == 0 else nc.vector
                        if eng is nc.scalar:
                            nc.scalar.copy(out=dst[:, m * P:(m + 1) * P], in_=pq[:])
                        else:
                            nc.vector.tensor_copy(out=dst[:, m * P:(m + 1) * P], in_=pq[:])

                pv = p256(F32)
                for kc in range(KD):
                    nc.tensor.matmul(out=pv[:], lhsT=h1T[:, kc * P:(kc + 1) * P],
                                     rhs=wv_sb[l][:, kc * D:(kc + 1) * D],
                                     start=(kc == 0), stop=(kc == KD - 1 and lnb_sb is None))
                if lnb_sb is not None:
                    nc.tensor.matmul(out=pv[:], lhsT=ones_rbf[:],
                                     rhs=lnb_sb[:, (2 * L + l) * D:(2 * L + l + 1) * D],
                                     start=False, stop=True)
                v_sb = sp.tile([P, D], BF16, tag="vsb")
                nc.scalar.copy(out=v_sb[:], in_=pv[:])

                den_ps = p256(F32)
                oT_ps = p256(F32)
                for h in range(H):
                    mq = h // 4
                    po = (h % 4) * DK
                    ps_s = p128(F32)
                    nc.tensor.matmul(
                        out=ps_s[:],
                        lhsT=kT[po:po + DK, mq * P:(mq + 1) * P],
                        rhs=qT[po:po + DK, mq * P:(mq + 1) * P],
                        start=True, stop=True, tile_position=(po, 0))
                    att = ap_.tile([P, P], BF16, tag="att")
                    nc.scalar.activation(out=att[:], in_=ps_s[:], func=AF.Exp,
                                         scale=ATT_SCALE)
                    # denominator, broadcast into the same (partition, free)
                    # slot oT_ps uses for this head's 32 d-rows
                    nc.tensor.matmul(
                        out=den_ps[po:po + DK, mq * P:(mq + 1) * P],
                        lhsT=ones_m32[:], rhs=att[:], start=True, stop=True,
                        tile_position=(0, po))
                    nc.tensor.matmul(
                        out=oT_ps[po:po + DK, mq * P:(mq + 1) * P],
                        lhsT=v_sb[:, h * DK:(h + 1) * DK],
                        rhs=att[:], start=True, stop=True,
                        tile_position=(0, po))

                rf_sb = sp.tile([P, D], F32, tag="rfsb")
                nc.vector.reciprocal(out=rf_sb[:], in_=den_ps[:])
                oTn = sp.tile([P, D], BF16, tag="oTn")
                nc.vector.tensor_tensor(out=oTn[:], in0=oT_ps[:], in1=rf_sb[:],
                                        op=ALU.mult)

                pxd = p256(F32)
                for kc in range(KD):
                    nc.tensor.matmul(out=pxd[:], lhsT=oTn[:, kc * P:(kc + 1) * P],
                                     rhs=wo_sb[l][:, kc * D:(kc + 1) * D],
                                     start=(kc == 0), stop=(kc == KD - 1))
                nc.vector.tensor_add(out=xb, in0=xb, in1=pxd[:])

                # ---------- FFN ----------
                h2 = sp.tile([P, D], BF16, tag="h2")
                ln_apply(xb, h2[:])
                h2T = sp.tile([P, D], BF16, tag="h2T")
                transpose_256(h2[:], h2T[:])

                a_sb = sp.tile([P, DI], BF16, tag="asb")
                for mi in range(KI):
                    pa = p128(F32)
                    for kc in range(KD):
                        nc.tensor.matmul(
                            out=pa[:],
                            lhsT=w1_sb[l][:, kc * DI + mi * P: kc * DI + (mi + 1) * P],
                            rhs=h2T[:, kc * P:(kc + 1) * P],
                            start=(kc == 0), stop=(kc == KD - 1))
                    nc.scalar.activation(out=a_sb[:, mi * P:(mi + 1) * P], in_=pa[:],
                                         func=AF.Relu,
                                         bias=b1t_sb[:, l * KI + mi: l * KI + mi + 1])

                pxd2 = p256(F32)
                for ki in range(KI):
                    nc.tensor.matmul(out=pxd2[:], lhsT=a_sb[:, ki * P:(ki + 1) * P],
                                     rhs=w2_sb[l][:, ki * D:(ki + 1) * D],
                                     start=(ki == 0), stop=False)
                nc.tensor.matmul(out=pxd2[:], lhsT=ones_rbf[:],
                                 rhs=b2r_sb[:, l * D:(l + 1) * D],
                                 start=False, stop=True)
                nc.vector.tensor_add(out=xb, in0=xb, in1=pxd2[:])

        # ================= final LN + head ==================================
        xl = cp.tile([BPC, D], F32)
        for b in range(BPC):
            nc.sync.dma_start(out=xl[b:b + 1, :],
                              in_=x_big[P - 1:P, b * D:(b + 1) * D])
        xls = cp.tile([BPC, D], BF16)
        ln_apply(xl[:], xls[:], extra_scale=onem32[:])   # scaled by (1-w)
        xlT = cp.tile([P, KD * BPC], BF16)
        for c in range(KD):
            pt = pmisc([P, BPC], BF16)
            nc.tensor.transpose(out=pt[:], in_=xls[:, c * P:(c + 1) * P],
                                identity=ident[0:BPC, 0:BPC])
            nc.scalar.copy(out=xlT[:, c * BPC:(c + 1) * BPC], in_=pt[:])

        for n in range(NVC):
            wt = hp.tile([P, KD * NV], BF16, tag="wout")
            for kc in range(KD):
                nc.sync.dma_start(out=wt[:, kc * NV:(kc + 1) * NV],
                                  in_=wout_d[kc * P:(kc + 1) * P, n * NV:(n + 1) * NV])
            bt = hp.tile([1, NV], BF16, tag="boutt")
            nc.sync.dma_start(out=bt[:], in_=bout_d[:, n * NV:(n + 1) * NV])
            plog = pmisc([BPC, NV])
            for kc in range(KD):
                nc.tensor.matmul(out=plog[:], lhsT=xlT[:, kc * BPC:(kc + 1) * BPC],
                                 rhs=wt[:, kc * NV:(kc + 1) * NV],
                                 start=(kc == 0), stop=False)
            nc.tensor.matmul(out=plog[:], lhsT=onemw_row[:],
                             rhs=bt[:], start=False, stop=True)
            lsb = hp.tile([BPC, NV], F32, tag="lsb")
            if n % 2 == 0:
                nc.scalar.copy(out=lsb[:], in_=plog[:])
            else:
                nc.vector.tensor_copy(out=lsb[:], in_=plog[:])
            di = nc.sync.dma_start(out=out_d[:, n * NV:(n + 1) * NV], in_=lsb[:])
            head_dma_insts.append(di.ins)

        # ================= boost RMW scatter-add ============================
        out_flat = out_d[:].rearrange("a v -> (a v)").rearrange("(n c) -> n c", c=BLK)
        for r in range(BCAP // P):
            bi = gp.tile([P, 1], I32, tag="bidx")
            nc.sync.dma_start(out=bi[:], in_=bidx_d[r * P:(r + 1) * P])
            br = gp.tile([P, BLK], F32, tag="brow")
            nc.sync.dma_start(out=br[:], in_=brows_d[r * P:(r + 1) * P, :])
            g = gp.tile([P, BLK], F32, tag="grmw")
            nc.vector.memset(g[:], 0.0)
            gi = nc.gpsimd.indirect_dma_start(
                out=g[:], out_offset=None, in_=out_flat,
                in_offset=IndirectOffsetOnAxis(ap=bi[:, :1], axis=0),
                bounds_check=NBLK - 1, oob_is_err=False)
            for di in head_dma_insts:
                tile.add_dep_helper(gi.ins, di, reason="boost RMW after head DMA")
            nc.vector.scalar_tensor_tensor(out=g[:], in0=br[:], scalar=sbc[:],
                                           in1=g[:], op0=ALU.mult, op1=ALU.add)
            nc.gpsimd.indirect_dma_start(
                out=out_flat, out_offset=IndirectOffsetOnAxis(ap=bi[:, :1], axis=0),
                in_=g[:], in_offset=None,
                bounds_check=NBLK - 1, oob_is_err=False)

    nc.compile()
    return nc


_CACHE = {}


def _get_nc(with_ln1_bias: bool):
    key = bool(with_ln1_bias)
    if key not in _CACHE:
        _CACHE[key] = _build(key)
    return _CACHE[key]


def _prep_inputs(inputs):
    """Host-side preprocessing: returns (in_maps, with_ln1_bias)."""
    f = lambda a: np.asarray(a, dtype=np.float32)
    bf = lambda a: np.ascontiguousarray(a).astype(ml_dtypes.bfloat16)

    locations = np.asarray(inputs["locations"]).astype(np.int64)
    users = np.asarray(inputs["users"]).astype(np.int64)
    loc_emb = f(inputs["loc_emb"])
    user_emb = f(inputs["user_emb"])
    Wq, Wk, Wv, Wo = (f(inputs[k]) for k in ("Wq", "Wk", "Wv", "Wo"))
    W1, W2 = f(inputs["W1"]), f(inputs["W2"])
    b1, b2 = f(inputs["b1"]), f(inputs["b2"])
    ln1_g, ln1_b = f(inputs["ln1_g"]), f(inputs["ln1_b"])
    ln2_g, ln2_b = f(inputs["ln2_g"]), f(inputs["ln2_b"])
    lnf_g, lnf_b = f(inputs["lnf_g"]), f(inputs["lnf_b"])
    W_out, b_out = f(inputs["W_out"]), f(inputs["b_out"])
    position_boost = f(inputs["position_boost"])
    return_strength = f(inputs["return_strength"]).reshape(1, 1)
    ensemble_weight = f(inputs["ensemble_weight"]).reshape(1, 1)

    # LN folds (exact linear algebra, done once on host)
    wq_e = ln1_g[:, :, None] * Wq
    wk_e = ln1_g[:, :, None] * Wk
    wv_e = ln1_g[:, :, None] * Wv
    w1_e = ln2_g[:, :, None] * W1
    b1_e = b1 + np.einsum("ld,ldj->lj", ln2_b, W1)
    wout_e = lnf_g[:, None] * W_out
    bout_e = b_out + lnf_b @ W_out
    qrow = np.einsum("ld,ldj->lj", ln1_b, Wq)
    krow = np.einsum("ld,ldj->lj", ln1_b, Wk)
    vrow = np.einsum("ld,ldj->lj", ln1_b, Wv)
    with_ln1_bias = bool(max(np.abs(qrow).max(), np.abs(krow).max(),
                             np.abs(vrow).max()) > 0)
    lnb_rows = np.stack([qrow, krow, vrow])[:, :, None, :]  # [3, L, 1, D]

    b1t = np.transpose(b1_e.reshape(L, KI, P), (0, 2, 1)).copy()  # [L, P, KI]
    posenc = _posenc()

    shared = {
        "lemb": loc_emb,
        "uemb": bf(user_emb),
        "posenc": posenc,
        "wq": bf(wq_e), "wk": bf(wk_e), "wv": bf(wv_e), "wo": bf(Wo),
        "w1": bf(w1_e), "w2": bf(W2),
        "b1t": b1t, "b2r": bf(b2[:, None, :]),
        "lnbrows": bf(lnb_rows),
        "wout": bf(wout_e), "bout": bf(bout_e[None, :]),
        "ens": ensemble_weight, "rstr": return_strength,
    }

    in_maps = []
    for c in range(NCORES):
        lc = locations[c * BPC:(c + 1) * BPC]            # [BPC, S]
        uc = users[c * BPC:(c + 1) * BPC]
        # boost rows: block -> 256-wide accumulation row
        rows = {}
        for b in range(BPC):
            for j in range(NB):
                col = int(lc[b, S - 1 - j])
                flat = b * V + col
                blk, off = flat // BLK, flat % BLK
                if blk not in rows:
                    rows[blk] = np.zeros(BLK, np.float32)
                rows[blk][off] += position_boost[j]
        bidx = np.full((BCAP, 1), 1 << 20, np.int32)
        brows = np.zeros((BCAP, BLK), np.float32)
        for i, (blk, row) in enumerate(sorted(rows.items())):
            bidx[i, 0] = blk
            brows[i] = row
        m = dict(shared)
        m["locs"] = lc.astype(np.int32)
        m["users"] = np.repeat(uc.astype(np.int32)[:, None], P, axis=1)
        m["bidx"] = bidx
        m["brows"] = brows
        in_maps.append(m)
    return in_maps, with_ln1_bias


def kernel(**inputs) -> np.ndarray:
    in_maps, with_ln1_bias = _prep_inputs(inputs)
    nc = _get_nc(with_ln1_bias)
    res = bass_utils.run_bass_kernel_spmd(nc, in_maps, core_ids=list(range(NCORES)))
    return np.concatenate([r["out"] for r in res.results], axis=0)
